# revision 4
# baseline (speedup 1.0000x reference)
import sys

if "/opt/trn_rl_repo" not in sys.path:
    sys.path.insert(0, "/opt/trn_rl_repo")

import numpy as np
import ml_dtypes

import concourse.bass as bass
import concourse.bacc as bacc
import concourse.tile as tile
import concourse.mybir as mybir
from concourse import bass_utils

# Problem shapes (nn_ChebConv): x (16, 12288), L (12288, 12288),
# weights (5, 16, 32), bias (32,). out (32, 12288).
C_IN = 16
C_OUT = 32
K_CHEB = 5
V = 12288
N_CORES = 8
VLOC = V // N_CORES          # 1536 columns of the V axis per core
P = 128                      # partition tile
NT_VC = V // P               # 96 contraction tiles per step
N_CH = VLOC // 512           # 3 psum chunks of 512
NB = 8                       # vc-tiles per bulk DMA (8*128 rows = 3 MB f32 / 1.5 MB bf16)

_CACHE: dict = {}


def _build(cfg: str):
    """Build + compile the SPMD bass kernel. cfg in {"bf16", "f32r", "f32"}."""
    if cfg == "bf16":
        mm_dt = mybir.dt.bfloat16
    elif cfg == "f32r":
        mm_dt = mybir.dt.float32r
    else:
        mm_dt = mybir.dt.float32
    f32 = mybir.dt.float32

    nc = bacc.Bacc("TRN2", target_bir_lowering=False, debug=False,
                   num_devices=N_CORES)

    # Per-core DRAM I/O.
    lt = nc.dram_tensor("lt", [V, VLOC], mm_dt, kind="ExternalInput")       # L^T[:, cols_d]
    xt = nc.dram_tensor("xt", [V, C_IN], mm_dt, kind="ExternalInput")       # x^T (replicated)
    xc = nc.dram_tensor("xc", [C_IN, VLOC], f32, kind="ExternalInput")      # x[:, cols_d]
    wf = nc.dram_tensor("wf", [K_CHEB * C_IN, C_OUT], f32, kind="ExternalInput")
    bias_in = nc.dram_tensor("bias_in", [C_OUT, 1], f32, kind="ExternalInput")
    id16 = nc.dram_tensor("id16", [C_IN, C_IN], f32, kind="ExternalInput")
    out = nc.dram_tensor("out", [C_OUT, VLOC], f32, kind="ExternalOutput")

    with tile.TileContext(nc) as tc:
        with (
            tc.tile_pool(name="ltp", bufs=4) as ltp,
            tc.tile_pool(name="persist", bufs=1) as persist,
            tc.tile_pool(name="skp", bufs=2) as skp,
            tc.tile_pool(name="work", bufs=2) as work,
            tc.tile_pool(name="acc", bufs=3, space="PSUM") as accp,
            tc.tile_pool(name="tpp", bufs=4, space="PSUM") as tpp,
            tc.tile_pool(name="dram", bufs=1, space="DRAM") as dram,
        ):
            # ---- persistent small tensors ----
            xt_sb = persist.tile([P, NT_VC * C_IN], mm_dt)
            nc.scalar.dma_start(
                xt_sb[:].rearrange("p (j c) -> p j c", j=NT_VC),
                xt.ap().rearrange("(j p) c -> p j c", p=P))
            xc_sb = persist.tile([C_IN, VLOC], f32)
            nc.scalar.dma_start(xc_sb[:], xc.ap())
            w_sb = persist.tile([C_IN, K_CHEB * C_OUT], f32)
            for k in range(K_CHEB):
                nc.scalar.dma_start(
                    w_sb[:, k * C_OUT:(k + 1) * C_OUT],
                    wf.ap()[k * C_IN:(k + 1) * C_IN, :])
            bias_sb = persist.tile([C_OUT, 1], f32)
            nc.scalar.dma_start(bias_sb[:], bias_in.ap())
            id_sb = persist.tile([C_IN, C_IN], f32)
            nc.scalar.dma_start(id_sb[:], id16.ap())

            # T_k in (C, VLOC) fp32 layout; t_tiles[0] is the x chunk.
            t1_sb = persist.tile([C_IN, VLOC], f32)
            t2_sb = persist.tile([C_IN, VLOC], f32)
            t3_sb = persist.tile([C_IN, VLOC], f32)
            t4_sb = persist.tile([C_IN, VLOC], f32)
            t_tiles = [xc_sb, t1_sb, t2_sb, t3_sb, t4_sb]

            sk_prev = xt_sb  # stationary for step 1: T_0^T = x^T
            lt_r = lt.ap().rearrange("(g u p) c -> g p u c", p=P, u=NB)

            for k in range(1, K_CHEB):
                # ---- psum[ch] = sum_vc S_prev[vc].T @ LT[vc, ch] ----
                acc = [accp.tile([C_IN, 512], f32, name=f"acc{k}_{ch}", tag="acc")
                       for ch in range(N_CH)]
                for g in range(NT_VC // NB):
                    lt_t = ltp.tile([P, NB * VLOC], mm_dt, name=f"lt{k}_{g}",
                                    tag="lt")
                    nc.sync.dma_start(
                        lt_t[:].rearrange("p (u c) -> p u c", u=NB), lt_r[g])
                    for u in range(NB):
                        j = g * NB + u
                        for ch in range(N_CH):
                            nc.tensor.matmul(
                                acc[ch][:],
                                lhsT=sk_prev[:, j * C_IN:(j + 1) * C_IN],
                                rhs=lt_t[:, u * VLOC + ch * 512:
                                         u * VLOC + (ch + 1) * 512],
                                start=(j == 0),
                                stop=(j == NT_VC - 1),
                            )
                # ---- T_k = 2*psum - T_{k-2}  (step 1: T_1 = psum) ----
                tk = t_tiles[k]
                for ch in range(N_CH):
                    sl = slice(ch * 512, (ch + 1) * 512)
                    if k == 1:
                        nc.vector.tensor_copy(tk[:, sl], acc[ch][:])
                    else:
                        nc.vector.scalar_tensor_tensor(
                            tk[:, sl], acc[ch][:], 2.0, t_tiles[k - 2][:, sl],
                            mybir.AluOpType.mult, mybir.AluOpType.subtract)

                if k < K_CHEB - 1:
                    # ---- local chunk of S_k = T_k^T, cast, all-gather ----
                    sc_stage = work.tile([P, (VLOC // P) * C_IN], mm_dt,
                                         name=f"scs{k}", tag="scs")
                    for j2 in range(VLOC // P):
                        tp_ps = tpp.tile([P, C_IN], f32, name=f"tp{k}_{j2}",
                                         tag="tp")
                        nc.tensor.transpose(
                            tp_ps[:], tk[:, j2 * P:(j2 + 1) * P], id_sb[:])
                        nc.vector.tensor_copy(
                            sc_stage[:, j2 * C_IN:(j2 + 1) * C_IN], tp_ps[:])
                    cc_in = dram.tile([VLOC, C_IN], mm_dt, name=f"ccin{k}")
                    cc_out = dram.tile([V, C_IN], mm_dt, name=f"ccout{k}")
                    nc.scalar.dma_start(
                        cc_in.rearrange("(j p) c -> p j c", p=P),
                        sc_stage[:].rearrange("p (j c) -> p j c", j=VLOC // P))
                    nc.gpsimd.collective_compute(
                        "AllGather",
                        mybir.AluOpType.bypass,
                        replica_groups=[list(range(N_CORES))],
                        ins=[cc_in.opt()],
                        outs=[cc_out.opt()],
                    )
                    sk_new = skp.tile([P, NT_VC * C_IN], mm_dt,
                                      name=f"sk{k}", tag="sk")
                    nc.scalar.dma_start(
                        sk_new[:].rearrange("p (j c) -> p j c", j=NT_VC),
                        cc_out.rearrange("(j p) c -> p j c", p=P))
                    sk_prev = sk_new

            # ---- out[o, v] = sum_k w_k^T @ T_k + bias ----
            for ch in range(N_CH):
                sl = slice(ch * 512, (ch + 1) * 512)
                ein = accp.tile([C_OUT, 512], f32, name=f"ein{ch}", tag="acc")
                for k in range(K_CHEB):
                    nc.tensor.matmul(
                        ein[:],
                        lhsT=w_sb[:, k * C_OUT:(k + 1) * C_OUT],
                        rhs=t_tiles[k][:, sl],
                        start=(k == 0),
                        stop=(k == K_CHEB - 1),
                    )
                res = work.tile([C_OUT, 512], f32, name=f"res{ch}", tag="res")
                nc.vector.tensor_scalar_add(res[:], ein[:], bias_sb[:])
                nc.scalar.dma_start(out.ap()[:, sl], res[:])

    nc.compile()
    return nc


def _prep_inputs(x, L, weights, bias, cfg: str):
    if cfg == "bf16":
        np_dt = ml_dtypes.bfloat16
    else:
        np_dt = np.float32
    x = np.asarray(x, dtype=np.float32)
    L = np.asarray(L, dtype=np.float32)
    weights = np.asarray(weights, dtype=np.float32)
    bias = np.asarray(bias, dtype=np.float32)

    Lt = np.ascontiguousarray(L.T).astype(np_dt)          # (V, V)
    xt = np.ascontiguousarray(x.T).astype(np_dt)          # (V, C_IN)
    wf = np.ascontiguousarray(weights.reshape(K_CHEB * C_IN, C_OUT))
    b = np.ascontiguousarray(bias.reshape(C_OUT, 1))
    id16 = np.eye(C_IN, dtype=np.float32)

    in_maps = []
    for d in range(N_CORES):
        cols = slice(d * VLOC, (d + 1) * VLOC)
        in_maps.append({
            "lt": np.ascontiguousarray(Lt[:, cols]),
            "xt": xt,
            "xc": np.ascontiguousarray(x[:, cols]),
            "wf": wf,
            "bias_in": b,
            "id16": id16,
        })
    return in_maps


def run(x, L, weights, bias, cfg: str = "bf16", trace: bool = False):
    if cfg not in _CACHE:
        _CACHE[cfg] = _build(cfg)
    nc = _CACHE[cfg]
    in_maps = _prep_inputs(x, L, weights, bias, cfg)
    res = bass_utils.run_bass_kernel_spmd(
        nc, in_maps, core_ids=list(range(N_CORES)), trace=trace)
    out = np.concatenate([res.results[d]["out"] for d in range(N_CORES)],
                         axis=1)
    return out.astype(np.float32), res


def kernel(x, L, weights, bias):
    out, _ = run(x, L, weights, bias, cfg="bf16")
    return out


# revision 15
# speedup vs baseline: 1.0570x; 1.0570x over previous
import sys

if "/opt/trn_rl_repo" not in sys.path:
    sys.path.insert(0, "/opt/trn_rl_repo")

import numpy as np
import ml_dtypes

import concourse.bass as bass
import concourse.bacc as bacc
import concourse.tile as tile
import concourse.mybir as mybir
from concourse import bass_utils

# Problem shapes (nn_ChebConv): x (16, 12288), L (12288, 12288),
# weights (5, 16, 32), bias (32,). out (32, 12288).
#
# Sharding: core d owns V-columns [d*1536, (d+1)*1536).  Host feeds each
# core lt = L^T[:, cols_d] (so the PE streams L with the contraction dim
# on partitions), row-interleaved within 1024-row groups so each SBUF
# partition reads one contiguous 24 KB chunk per bulk DMA.
#
# Recurrence per step k: psum(16,512)x3 accumulates T_{k-1} @ L^T over 96
# vc-tiles (stationary = all-gathered T_{k-1}^T tiles (128,16), moving =
# lt tiles (128,512) bf16); one DVE op forms T_k = 2*psum - T_{k-2}; the
# local (16,1536) chunk is PE-transposed, cast to bf16 and all-gathered
# for the next step's stationary.  The first RES_T vc-tiles of lt stay
# resident in SBUF across all 4 steps (read once instead of 4x).
C_IN = 16
C_OUT = 32
K_CHEB = 5
V = 12288
N_CORES = 8
VLOC = V // N_CORES          # 1536 columns of the V axis per core
P = 128
NT_VC = V // P               # 96 contraction tiles per step
N_CH = VLOC // 512           # 3 psum chunks of 512
NB = 8                       # vc-tiles per bulk DMA (1024 rows)
NG = NT_VC // NB             # 12 groups
RES_T = 16                   # vc-tiles resident in SBUF (of 96)
RES_G = RES_T // NB          # resident groups
LT_BUFS = 4

_CACHE: dict = {}


def _build(cfg: str):
    if cfg == "bf16":
        mm_dt = mybir.dt.bfloat16
    elif cfg == "f32r":
        mm_dt = mybir.dt.float32r
    else:
        mm_dt = mybir.dt.float32
    f32 = mybir.dt.float32

    res_g = RES_G if cfg == "bf16" else 0

    nc = bacc.Bacc("TRN2", target_bir_lowering=False, debug=False,
                   num_devices=N_CORES)

    lt = nc.dram_tensor("lt", [V, VLOC], mm_dt, kind="ExternalInput")
    xt = nc.dram_tensor("xt", [V, C_IN], mm_dt, kind="ExternalInput")
    xc = nc.dram_tensor("xc", [C_IN, VLOC], f32, kind="ExternalInput")
    wf = nc.dram_tensor("wf", [K_CHEB * C_IN, C_OUT], f32, kind="ExternalInput")
    bias_in = nc.dram_tensor("bias_in", [C_OUT, 1], f32, kind="ExternalInput")
    id16 = nc.dram_tensor("id16", [C_IN, C_IN], f32, kind="ExternalInput")
    out = nc.dram_tensor("out", [C_OUT, VLOC], f32, kind="ExternalOutput")

    lt_r = lt.ap().rearrange("(g p u) c -> g p u c", p=P, u=NB)

    with tile.TileContext(nc) as tc:
        with (
            tc.tile_pool(name="ltp", bufs=LT_BUFS) as ltp,
            tc.tile_pool(name="persist", bufs=1) as persist,
            tc.tile_pool(name="stat", bufs=2 * NG + NG) as statp,
            tc.tile_pool(name="work", bufs=2) as work,
            tc.tile_pool(name="acc", bufs=3, space="PSUM") as accp,
            tc.tile_pool(name="tpp", bufs=4, space="PSUM") as tpp,
            tc.tile_pool(name="dram", bufs=1, space="DRAM") as dram,
        ):
            # ---- persistent small tensors ----
            w_sb = persist.tile([C_IN, K_CHEB * C_OUT], f32)
            for k in range(K_CHEB):
                nc.scalar.dma_start(
                    w_sb[:, k * C_OUT:(k + 1) * C_OUT],
                    wf.ap()[k * C_IN:(k + 1) * C_IN, :])
            bias_sb = persist.tile([C_OUT, 1], f32)
            nc.scalar.dma_start(bias_sb[:], bias_in.ap())
            id_sb = persist.tile([C_IN, C_IN], f32)
            nc.scalar.dma_start(id_sb[:], id16.ap())

            # T_k in (C, VLOC) fp32 layout, all at partition base 0
            xc_sb = persist.tile([C_IN, VLOC], f32)
            nc.scalar.dma_start(xc_sb[:], xc.ap())
            t1_sb = persist.tile([C_IN, VLOC], f32)
            t2_sb = persist.tile([C_IN, VLOC], f32)
            t3_sb = persist.tile([C_IN, VLOC], f32)
            t4_sb = persist.tile([C_IN, VLOC], f32)
            t_tiles = [xc_sb, t1_sb, t2_sb, t3_sb, t4_sb]

            # step-1 stationary: x^T as 12 per-group tiles (128, NB*16)
            def load_stat(src_r, tag_k):
                tiles = []
                for g in range(NG):
                    s = statp.tile([P, NB * C_IN], mm_dt,
                                   name=f"st{tag_k}_{g}", tag="stat")
                    nc.scalar.dma_start(
                        s[:].rearrange("p (j c) -> p j c", j=NB), src_r[g])
                    tiles.append(s)
                return tiles

            xt_r = xt.ap().rearrange("(g j p) c -> g p j c", p=P, j=NB)
            sk_tiles = load_stat(xt_r, 0)

            # resident lt: first RES_T vc-tiles, loaded once
            if res_g:
                rs_sb = persist.tile([P, RES_T * VLOC], mm_dt)
                for g in range(res_g):
                    nc.sync.dma_start(
                        rs_sb[:, g * NB * VLOC:(g + 1) * NB * VLOC]
                        .rearrange("p (u c) -> p u c", u=NB),
                        lt_r[g])

            for k in range(1, K_CHEB):
                acc = [accp.tile([C_IN, 512], f32, name=f"acc{k}_{ch}",
                                 tag="acc") for ch in range(N_CH)]
                for g in range(NG):
                    if g < res_g:
                        src = rs_sb
                        base = g * NB * VLOC
                    else:
                        src = ltp.tile([P, NB * VLOC], mm_dt,
                                       name=f"lt{k}_{g}", tag="lt")
                        nc.sync.dma_start(
                            src[:].rearrange("p (u c) -> p u c", u=NB),
                            lt_r[g])
                        base = 0
                    for u in range(NB):
                        j = g * NB + u
                        st = sk_tiles[g]
                        for ch in range(N_CH):
                            nc.tensor.matmul(
                                acc[ch][:],
                                lhsT=st[:, u * C_IN:(u + 1) * C_IN],
                                rhs=src[:, base + u * VLOC + ch * 512:
                                        base + u * VLOC + (ch + 1) * 512],
                                start=(j == 0),
                                stop=(j == NT_VC - 1),
                            )
                # ---- T_k = 2*psum - T_{k-2}  (step 1: T_1 = psum) ----
                tk = t_tiles[k]
                for ch in range(N_CH):
                    sl = slice(ch * 512, (ch + 1) * 512)
                    if k == 1:
                        nc.vector.tensor_copy(tk[:, sl], acc[ch][:])
                    else:
                        nc.vector.scalar_tensor_tensor(
                            tk[:, sl], acc[ch][:], 2.0,
                            t_tiles[k - 2][:, sl],
                            mybir.AluOpType.mult, mybir.AluOpType.subtract)

                if k < K_CHEB - 1:
                    # ---- transpose local chunk, cast, all-gather ----
                    sc_stage = work.tile([P, (VLOC // P) * C_IN], mm_dt,
                                         name=f"scs{k}", tag="scs")
                    for j2 in range(VLOC // P):
                        tp_ps = tpp.tile([P, C_IN], f32, name=f"tp{k}_{j2}",
                                         tag="tp")
                        nc.tensor.transpose(
                            tp_ps[:], tk[:, j2 * P:(j2 + 1) * P], id_sb[:])
                        nc.vector.tensor_copy(
                            sc_stage[:, j2 * C_IN:(j2 + 1) * C_IN], tp_ps[:])
                    cc_in = dram.tile([VLOC, C_IN], mm_dt, name=f"ccin{k}")
                    cc_out = dram.tile([V, C_IN], mm_dt, name=f"ccout{k}")
                    nc.scalar.dma_start(
                        cc_in.rearrange("(j p) c -> p j c", p=P),
                        sc_stage[:].rearrange("p (j c) -> p j c",
                                              j=VLOC // P))
                    nc.gpsimd.collective_compute(
                        "AllGather",
                        mybir.AluOpType.bypass,
                        replica_groups=[list(range(N_CORES))],
                        ins=[cc_in.opt()],
                        outs=[cc_out.opt()],
                    )
                    cc_r = cc_out.rearrange("(g j p) c -> g p j c", p=P, j=NB)
                    sk_tiles = load_stat(cc_r, k)

            # ---- out[o, v] = sum_k w_k^T @ T_k + bias ----
            for ch in range(N_CH):
                sl = slice(ch * 512, (ch + 1) * 512)
                ein = accp.tile([C_OUT, 512], f32, name=f"ein{ch}", tag="acc")
                for k in range(K_CHEB):
                    nc.tensor.matmul(
                        ein[:],
                        lhsT=w_sb[:, k * C_OUT:(k + 1) * C_OUT],
                        rhs=t_tiles[k][:, sl],
                        start=(k == 0),
                        stop=(k == K_CHEB - 1),
                    )
                res = work.tile([C_OUT, 512], f32, name=f"res{ch}", tag="res")
                nc.vector.tensor_scalar_add(res[:], ein[:], bias_sb[:])
                nc.scalar.dma_start(out.ap()[:, sl], res[:])

    nc.compile()
    return nc


def _interleave_rows(a):
    """Within each 1024-row group, reorder rows so row g*1024+8p+u holds
    original row g*1024+u*128+p (one contiguous 24 KB read per partition)."""
    ng = a.shape[0] // (P * NB)
    return np.ascontiguousarray(
        a.reshape(ng, NB, P, a.shape[1]).transpose(0, 2, 1, 3)
        .reshape(a.shape))


def _prep_inputs(x, L, weights, bias, cfg: str):
    np_dt = ml_dtypes.bfloat16 if cfg == "bf16" else np.float32
    x = np.asarray(x, dtype=np.float32)
    L = np.asarray(L, dtype=np.float32)
    weights = np.asarray(weights, dtype=np.float32)
    bias = np.asarray(bias, dtype=np.float32)

    Lt = np.ascontiguousarray(L.T).astype(np_dt)          # (V, V)
    xt = np.ascontiguousarray(x.T).astype(np_dt)          # (V, C_IN)

    wf = np.ascontiguousarray(weights.reshape(K_CHEB * C_IN, C_OUT))
    b_ = np.ascontiguousarray(bias.reshape(C_OUT, 1))
    id16 = np.eye(C_IN, dtype=np.float32)

    in_maps = []
    for d in range(N_CORES):
        cols = slice(d * VLOC, (d + 1) * VLOC)
        in_maps.append({
            "lt": _interleave_rows(np.ascontiguousarray(Lt[:, cols])),
            "xt": xt,
            "xc": np.ascontiguousarray(x[:, cols]),
            "wf": wf,
            "bias_in": b_,
            "id16": id16,
        })
    return in_maps


def run(x, L, weights, bias, cfg: str = "bf16", trace: bool = False):
    if cfg not in _CACHE:
        _CACHE[cfg] = _build(cfg)
    nc = _CACHE[cfg]
    in_maps = _prep_inputs(x, L, weights, bias, cfg)
    res = bass_utils.run_bass_kernel_spmd(
        nc, in_maps, core_ids=list(range(N_CORES)), trace=trace)
    out = np.concatenate([res.results[d]["out"] for d in range(N_CORES)],
                         axis=1)
    return out.astype(np.float32), res


def kernel(x, L, weights, bias):
    out, _ = run(x, L, weights, bias, cfg="bf16")
    return out


# revision 17
# speedup vs baseline: 1.1479x; 1.0861x over previous
import sys

if "/opt/trn_rl_repo" not in sys.path:
    sys.path.insert(0, "/opt/trn_rl_repo")

import numpy as np
import ml_dtypes

import concourse.bass as bass
import concourse.bacc as bacc
import concourse.tile as tile
import concourse.mybir as mybir
from concourse import bass_utils

# Problem shapes (nn_ChebConv): x (16, 12288), L (12288, 12288),
# weights (5, 16, 32), bias (32,). out (32, 12288).
#
# Sharding: core d owns V-columns [d*1536, (d+1)*1536).  Host feeds each
# core lt = L^T[:, cols_d] (so the PE streams L with the contraction dim
# on partitions), row-interleaved within 1024-row groups so each SBUF
# partition reads one contiguous 24 KB chunk per bulk DMA.
#
# Recurrence per step k: psum(16,512)x3 accumulates T_{k-1} @ L^T over 96
# vc-tiles (stationary = all-gathered T_{k-1}^T tiles (128,16), moving =
# lt tiles (128,512) bf16); one DVE op forms T_k = 2*psum - T_{k-2}; the
# local (16,1536) chunk is PE-transposed, cast to bf16 and all-gathered
# for the next step's stationary.  The first RES_T vc-tiles of lt stay
# resident in SBUF across all 4 steps (read once instead of 4x).
C_IN = 16
C_OUT = 32
K_CHEB = 5
V = 12288
N_CORES = 8
VLOC = V // N_CORES          # 1536 columns of the V axis per core
P = 128
NT_VC = V // P               # 96 contraction tiles per step
N_CH = VLOC // 512           # 3 psum chunks of 512
NB = 8                       # vc-tiles per bulk DMA (1024 rows)
NG = NT_VC // NB             # 12 groups
RES_T = 16                   # vc-tiles resident in SBUF (of 96)
RES_G = RES_T // NB          # resident groups
LT_BUFS = 4

_CACHE: dict = {}


def _build(cfg: str):
    if cfg == "bf16":
        mm_dt = mybir.dt.bfloat16
    elif cfg == "f32r":
        mm_dt = mybir.dt.float32r
    else:
        mm_dt = mybir.dt.float32
    f32 = mybir.dt.float32

    res_g = RES_G if cfg == "bf16" else 0

    nc = bacc.Bacc("TRN2", target_bir_lowering=False, debug=False,
                   num_devices=N_CORES)

    lt = nc.dram_tensor("lt", [V, VLOC], mm_dt, kind="ExternalInput")
    xt = nc.dram_tensor("xt", [V, C_IN], mm_dt, kind="ExternalInput")
    xc = nc.dram_tensor("xc", [C_IN, VLOC], f32, kind="ExternalInput")
    wf = nc.dram_tensor("wf", [K_CHEB * C_IN, C_OUT], f32, kind="ExternalInput")
    bias_in = nc.dram_tensor("bias_in", [C_OUT, 1], f32, kind="ExternalInput")
    id16 = nc.dram_tensor("id16", [C_IN, C_IN], f32, kind="ExternalInput")
    out = nc.dram_tensor("out", [C_OUT, VLOC], f32, kind="ExternalOutput")

    lt_r = lt.ap().rearrange("(g p u) c -> g p u c", p=P, u=NB)

    with tile.TileContext(nc) as tc:
        with (
            tc.tile_pool(name="ltp", bufs=LT_BUFS) as ltp,
            tc.tile_pool(name="persist", bufs=1) as persist,
            tc.tile_pool(name="stat", bufs=2 * NG + NG) as statp,
            tc.tile_pool(name="work", bufs=2) as work,
            tc.tile_pool(name="acc", bufs=3, space="PSUM") as accp,
            tc.tile_pool(name="tpp", bufs=4, space="PSUM") as tpp,
            tc.tile_pool(name="dram", bufs=1, space="DRAM") as dram,
        ):
            # ---- persistent small tensors ----
            w_sb = persist.tile([C_IN, K_CHEB * C_OUT], f32)
            for k in range(K_CHEB):
                nc.scalar.dma_start(
                    w_sb[:, k * C_OUT:(k + 1) * C_OUT],
                    wf.ap()[k * C_IN:(k + 1) * C_IN, :])
            bias_sb = persist.tile([C_OUT, 1], f32)
            nc.scalar.dma_start(bias_sb[:], bias_in.ap())
            id_sb = persist.tile([C_IN, C_IN], f32)
            nc.scalar.dma_start(id_sb[:], id16.ap())

            # T_k in (C, VLOC) fp32 layout, all at partition base 0
            xc_sb = persist.tile([C_IN, VLOC], f32)
            nc.scalar.dma_start(xc_sb[:], xc.ap())
            t1_sb = persist.tile([C_IN, VLOC], f32)
            t2_sb = persist.tile([C_IN, VLOC], f32)
            t3_sb = persist.tile([C_IN, VLOC], f32)
            t4_sb = persist.tile([C_IN, VLOC], f32)
            t_tiles = [xc_sb, t1_sb, t2_sb, t3_sb, t4_sb]

            # step-1 stationary: x^T as 12 per-group tiles (128, NB*16)
            def load_stat(src_r, tag_k):
                tiles = []
                for g in range(NG):
                    s = statp.tile([P, NB * C_IN], mm_dt,
                                   name=f"st{tag_k}_{g}", tag="stat")
                    nc.scalar.dma_start(
                        s[:].rearrange("p (j c) -> p j c", j=NB), src_r[g])
                    tiles.append(s)
                return tiles

            xt_r = xt.ap().rearrange("(g j p) c -> g p j c", p=P, j=NB)
            sk_tiles = load_stat(xt_r, 0)

            # tiny warm-up AllGather: pays the ~70us first-collective cost
            # concurrently with step 1 instead of on the critical path
            wu_in = dram.tile([P, C_IN], mm_dt, name="wu_in")
            wu_out = dram.tile([P * N_CORES, C_IN], mm_dt, name="wu_out")
            wu_sb = work.tile([P, C_IN], mm_dt, name="wu_sb", tag="scs")
            nc.vector.memset(wu_sb[:], 0.0)
            nc.scalar.dma_start(wu_in[:], wu_sb[:])
            nc.gpsimd.collective_compute(
                "AllGather",
                mybir.AluOpType.bypass,
                replica_groups=[list(range(N_CORES))],
                ins=[wu_in.opt()],
                outs=[wu_out.opt()],
            )

            # resident lt: LAST RES_T vc-tiles, loaded once.  Sitting at the
            # end of each step, their matmuls need no DMA — covering the
            # transpose/all-gather chain while lt prefetches the next step.
            if res_g:
                rs_sb = persist.tile([P, RES_T * VLOC], mm_dt)
                for i, g in enumerate(range(NG - res_g, NG)):
                    nc.sync.dma_start(
                        rs_sb[:, i * NB * VLOC:(i + 1) * NB * VLOC]
                        .rearrange("p (u c) -> p u c", u=NB),
                        lt_r[g])

            for k in range(1, K_CHEB):
                acc = [accp.tile([C_IN, 512], f32, name=f"acc{k}_{ch}",
                                 tag="acc") for ch in range(N_CH)]
                for g in range(NG):
                    if g >= NG - res_g:
                        src = rs_sb
                        base = (g - (NG - res_g)) * NB * VLOC
                    else:
                        src = ltp.tile([P, NB * VLOC], mm_dt,
                                       name=f"lt{k}_{g}", tag="lt")
                        nc.sync.dma_start(
                            src[:].rearrange("p (u c) -> p u c", u=NB),
                            lt_r[g])
                        base = 0
                    for u in range(NB):
                        j = g * NB + u
                        st = sk_tiles[g]
                        for ch in range(N_CH):
                            nc.tensor.matmul(
                                acc[ch][:],
                                lhsT=st[:, u * C_IN:(u + 1) * C_IN],
                                rhs=src[:, base + u * VLOC + ch * 512:
                                        base + u * VLOC + (ch + 1) * 512],
                                start=(j == 0),
                                stop=(j == NT_VC - 1),
                            )
                # ---- T_k = 2*psum - T_{k-2}  (step 1: T_1 = psum) ----
                tk = t_tiles[k]
                for ch in range(N_CH):
                    sl = slice(ch * 512, (ch + 1) * 512)
                    if k == 1:
                        nc.vector.tensor_copy(tk[:, sl], acc[ch][:])
                    else:
                        nc.vector.scalar_tensor_tensor(
                            tk[:, sl], acc[ch][:], 2.0,
                            t_tiles[k - 2][:, sl],
                            mybir.AluOpType.mult, mybir.AluOpType.subtract)

                if k < K_CHEB - 1:
                    # ---- transpose local chunk, cast, all-gather ----
                    sc_stage = work.tile([P, (VLOC // P) * C_IN], mm_dt,
                                         name=f"scs{k}", tag="scs")
                    for j2 in range(VLOC // P):
                        tp_ps = tpp.tile([P, C_IN], f32, name=f"tp{k}_{j2}",
                                         tag="tp")
                        nc.tensor.transpose(
                            tp_ps[:], tk[:, j2 * P:(j2 + 1) * P], id_sb[:])
                        nc.vector.tensor_copy(
                            sc_stage[:, j2 * C_IN:(j2 + 1) * C_IN], tp_ps[:])
                    cc_in = dram.tile([VLOC, C_IN], mm_dt, name=f"ccin{k}")
                    cc_out = dram.tile([V, C_IN], mm_dt, name=f"ccout{k}")
                    nc.scalar.dma_start(
                        cc_in.rearrange("(j p) c -> p j c", p=P),
                        sc_stage[:].rearrange("p (j c) -> p j c",
                                              j=VLOC // P))
                    nc.gpsimd.collective_compute(
                        "AllGather",
                        mybir.AluOpType.bypass,
                        replica_groups=[list(range(N_CORES))],
                        ins=[cc_in.opt()],
                        outs=[cc_out.opt()],
                    )
                    cc_r = cc_out.rearrange("(g j p) c -> g p j c", p=P, j=NB)
                    sk_tiles = load_stat(cc_r, k)

            # ---- out[o, v] = sum_k w_k^T @ T_k + bias ----
            for ch in range(N_CH):
                sl = slice(ch * 512, (ch + 1) * 512)
                ein = accp.tile([C_OUT, 512], f32, name=f"ein{ch}", tag="acc")
                for k in range(K_CHEB):
                    nc.tensor.matmul(
                        ein[:],
                        lhsT=w_sb[:, k * C_OUT:(k + 1) * C_OUT],
                        rhs=t_tiles[k][:, sl],
                        start=(k == 0),
                        stop=(k == K_CHEB - 1),
                    )
                res = work.tile([C_OUT, 512], f32, name=f"res{ch}", tag="res")
                nc.vector.tensor_scalar_add(res[:], ein[:], bias_sb[:])
                nc.scalar.dma_start(out.ap()[:, sl], res[:])

    nc.compile()
    return nc


def _interleave_rows(a):
    """Within each 1024-row group, reorder rows so row g*1024+8p+u holds
    original row g*1024+u*128+p (one contiguous 24 KB read per partition)."""
    ng = a.shape[0] // (P * NB)
    return np.ascontiguousarray(
        a.reshape(ng, NB, P, a.shape[1]).transpose(0, 2, 1, 3)
        .reshape(a.shape))


def _prep_inputs(x, L, weights, bias, cfg: str):
    np_dt = ml_dtypes.bfloat16 if cfg == "bf16" else np.float32
    x = np.asarray(x, dtype=np.float32)
    L = np.asarray(L, dtype=np.float32)
    weights = np.asarray(weights, dtype=np.float32)
    bias = np.asarray(bias, dtype=np.float32)

    Lt = np.ascontiguousarray(L.T).astype(np_dt)          # (V, V)
    xt = np.ascontiguousarray(x.T).astype(np_dt)          # (V, C_IN)

    wf = np.ascontiguousarray(weights.reshape(K_CHEB * C_IN, C_OUT))
    b_ = np.ascontiguousarray(bias.reshape(C_OUT, 1))
    id16 = np.eye(C_IN, dtype=np.float32)

    in_maps = []
    for d in range(N_CORES):
        cols = slice(d * VLOC, (d + 1) * VLOC)
        in_maps.append({
            "lt": _interleave_rows(np.ascontiguousarray(Lt[:, cols])),
            "xt": xt,
            "xc": np.ascontiguousarray(x[:, cols]),
            "wf": wf,
            "bias_in": b_,
            "id16": id16,
        })
    return in_maps


def run(x, L, weights, bias, cfg: str = "bf16", trace: bool = False):
    if cfg not in _CACHE:
        _CACHE[cfg] = _build(cfg)
    nc = _CACHE[cfg]
    in_maps = _prep_inputs(x, L, weights, bias, cfg)
    res = bass_utils.run_bass_kernel_spmd(
        nc, in_maps, core_ids=list(range(N_CORES)), trace=trace)
    out = np.concatenate([res.results[d]["out"] for d in range(N_CORES)],
                         axis=1)
    return out.astype(np.float32), res


def kernel(x, L, weights, bias):
    out, _ = run(x, L, weights, bias, cfg="bf16")
    return out


# revision 27
# speedup vs baseline: 1.2022x; 1.0473x over previous
import sys

if "/opt/trn_rl_repo" not in sys.path:
    sys.path.insert(0, "/opt/trn_rl_repo")

import numpy as np
import ml_dtypes

import concourse.bass as bass
import concourse.bacc as bacc
import concourse.tile as tile
import concourse.mybir as mybir
from concourse import bass_utils

# Problem shapes (nn_ChebConv): x (16, 12288), L (12288, 12288),
# weights (5, 16, 32), bias (32,). out (32, 12288).
#
# Sharding: core d owns V-columns [d*1536, (d+1)*1536).  Host feeds each
# core lt = L^T[:, cols_d] (so the PE streams L with the contraction dim
# on partitions), row-interleaved within 1024-row groups so each SBUF
# partition reads one contiguous 24 KB chunk per bulk DMA.
#
# Recurrence per step k: psum(16,512)x3 accumulates T_{k-1} @ L^T over 96
# vc-tiles (stationary = all-gathered T_{k-1}^T tiles (128,16), moving =
# lt tiles (128,512) bf16); one DVE op forms T_k = 2*psum - T_{k-2}; the
# local (16,1536) chunk is PE-transposed, cast to bf16 and all-gathered
# for the next step's stationary.  The first RES_T vc-tiles of lt stay
# resident in SBUF across all 4 steps (read once instead of 4x).
C_IN = 16
C_OUT = 32
K_CHEB = 5
V = 12288
N_CORES = 8
VLOC = V // N_CORES          # 1536 columns of the V axis per core
P = 128
NT_VC = V // P               # 96 contraction tiles per step
N_CH = VLOC // 512           # 3 psum chunks of 512
NB = 8                       # vc-tiles per bulk lt DMA (1024 rows)
NG = NT_VC // NB             # 12 groups
RES_T = 16                   # vc-tiles resident in SBUF (of 96)
RES_G = RES_T // NB          # resident groups
LT_BUFS = 4
NB_S = VLOC // P             # stationary group: 12 vc-tiles = one rank chunk
NG_S = V // (P * NB_S)       # 8 stationary groups

_CACHE: dict = {}


def _build(cfg: str):
    if cfg == "bf16":
        mm_dt = mybir.dt.bfloat16
    elif cfg == "f32r":
        mm_dt = mybir.dt.float32r
    else:
        mm_dt = mybir.dt.float32
    f32 = mybir.dt.float32

    res_g = RES_G if cfg == "bf16" else 0

    nc = bacc.Bacc("TRN2", target_bir_lowering=False, debug=False,
                   num_devices=N_CORES)

    lt = nc.dram_tensor("lt", [V, VLOC], mm_dt, kind="ExternalInput")
    xt = nc.dram_tensor("xt", [V, C_IN], mm_dt, kind="ExternalInput")
    xc = nc.dram_tensor("xc", [C_IN, VLOC], f32, kind="ExternalInput")
    wf = nc.dram_tensor("wf", [K_CHEB * C_IN, C_OUT], f32, kind="ExternalInput")
    bias_in = nc.dram_tensor("bias_in", [C_OUT, 1], f32, kind="ExternalInput")
    id16 = nc.dram_tensor("id16", [C_IN, C_IN], f32, kind="ExternalInput")
    out = nc.dram_tensor("out", [C_OUT, VLOC], f32, kind="ExternalOutput")

    lt_r = lt.ap().rearrange("(g p u) c -> g p u c", p=P, u=NB)

    with tile.TileContext(nc) as tc:
        with (
            tc.tile_pool(name="ltp", bufs=LT_BUFS) as ltp,
            tc.tile_pool(name="persist", bufs=1) as persist,
            tc.tile_pool(name="stat", bufs=3 * NG_S) as statp,
            tc.tile_pool(name="work", bufs=2) as work,
            tc.tile_pool(name="acc", bufs=3, space="PSUM") as accp,
            tc.tile_pool(name="tpp", bufs=4, space="PSUM") as tpp,
            tc.tile_pool(name="dram", bufs=1, space="DRAM") as dram,
        ):
            # ---- persistent small tensors ----
            w_sb = persist.tile([C_IN, K_CHEB * C_OUT], f32)
            for k in range(K_CHEB):
                nc.scalar.dma_start(
                    w_sb[:, k * C_OUT:(k + 1) * C_OUT],
                    wf.ap()[k * C_IN:(k + 1) * C_IN, :])
            bias_sb = persist.tile([C_OUT, 1], f32)
            nc.scalar.dma_start(bias_sb[:], bias_in.ap())
            id_sb = persist.tile([C_IN, C_IN], f32)
            nc.scalar.dma_start(id_sb[:], id16.ap())

            # T_k in (C, VLOC) fp32 layout, all at partition base 0
            xc_sb = persist.tile([C_IN, VLOC], f32)
            nc.scalar.dma_start(xc_sb[:], xc.ap())
            t1_sb = persist.tile([C_IN, VLOC], f32)
            t2_sb = persist.tile([C_IN, VLOC], f32)
            t3_sb = persist.tile([C_IN, VLOC], f32)
            t4_sb = persist.tile([C_IN, VLOC], f32)
            t_tiles = [xc_sb, t1_sb, t2_sb, t3_sb, t4_sb]

            # stationary tensors (x^T, then each gathered T_k^T) live in
            # rank-aligned 1536-row groups, row-interleaved so partition p
            # reads rows [g*1536 + 12p, +12) — one 384 B chunk.
            def load_stat(src_r, tag_k):
                tiles = []
                for g in range(NG_S):
                    s = statp.tile([P, NB_S * C_IN], mm_dt,
                                   name=f"st{tag_k}_{g}", tag="stat")
                    nc.scalar.dma_start(
                        s[:].rearrange("p (j c) -> p j c", j=NB_S), src_r[g])
                    tiles.append(s)
                return tiles

            xt_r = xt.ap().rearrange("(g p j) c -> g p j c", p=P, j=NB_S)
            sk_tiles = load_stat(xt_r, 0)

            # tiny warm-up AllGather: pays the ~70us first-collective cost
            # concurrently with step 1 instead of on the critical path
            wu_sb = work.tile([P, C_IN], mm_dt, name="wu_sb", tag="wu")
            nc.vector.memset(wu_sb[:], 0.0)
            for w in range(2):
                wu_in = dram.tile([P, C_IN], mm_dt, name=f"wu_in{w}")
                wu_out = dram.tile([P * N_CORES, C_IN], mm_dt,
                                   name=f"wu_out{w}")
                nc.scalar.dma_start(wu_in[:], wu_sb[:])
                nc.gpsimd.collective_compute(
                    "AllGather",
                    mybir.AluOpType.bypass,
                    replica_groups=[list(range(N_CORES))],
                    ins=[wu_in.opt()],
                    outs=[wu_out.opt()],
                )

            # resident lt: LAST RES_T vc-tiles, loaded once.  Sitting at the
            # end of each step, their matmuls need no DMA — covering the
            # transpose/all-gather chain while lt prefetches the next step.
            if res_g:
                rs_sb = persist.tile([P, RES_T * VLOC], mm_dt)
                for i, g in enumerate(range(NG - res_g, NG)):
                    nc.sync.dma_start(
                        rs_sb[:, i * NB * VLOC:(i + 1) * NB * VLOC]
                        .rearrange("p (u c) -> p u c", u=NB),
                        lt_r[g])

            for k in range(1, K_CHEB):
                acc = [accp.tile([C_IN, 512], f32, name=f"acc{k}_{ch}",
                                 tag="acc") for ch in range(N_CH)]
                for g in range(NG):
                    if g >= NG - res_g:
                        src = rs_sb
                        base = (g - (NG - res_g)) * NB * VLOC
                    else:
                        src = ltp.tile([P, NB * VLOC], mm_dt,
                                       name=f"lt{k}_{g}", tag="lt")
                        nc.sync.dma_start(
                            src[:].rearrange("p (u c) -> p u c", u=NB),
                            lt_r[g])
                        base = 0
                    for u in range(NB):
                        j = g * NB + u
                        st = sk_tiles[j // NB_S]
                        us = j % NB_S
                        for ch in range(N_CH):
                            nc.tensor.matmul(
                                acc[ch][:],
                                lhsT=st[:, us * C_IN:(us + 1) * C_IN],
                                rhs=src[:, base + u * VLOC + ch * 512:
                                        base + u * VLOC + (ch + 1) * 512],
                                start=(j == 0),
                                stop=(j == NT_VC - 1),
                            )
                # ---- T_k = 2*psum - T_{k-2}  (step 1: T_1 = psum) ----
                tk = t_tiles[k]
                for ch in range(N_CH):
                    sl = slice(ch * 512, (ch + 1) * 512)
                    if k == 1:
                        nc.vector.tensor_copy(tk[:, sl], acc[ch][:])
                    else:
                        nc.vector.scalar_tensor_tensor(
                            tk[:, sl], acc[ch][:], 2.0,
                            t_tiles[k - 2][:, sl],
                            mybir.AluOpType.mult, mybir.AluOpType.subtract)

                if k < K_CHEB - 1:
                    # ---- transpose local chunk, cast, all-gather ----
                    sc_stage = work.tile([P, (VLOC // P) * C_IN], mm_dt,
                                         name=f"scs{k}", tag="scs")
                    for j2 in range(VLOC // P):
                        tp_ps = tpp.tile([P, C_IN], f32, name=f"tp{k}_{j2}",
                                         tag="tp")
                        nc.tensor.transpose(
                            tp_ps[:], tk[:, j2 * P:(j2 + 1) * P], id_sb[:])
                        nc.vector.tensor_copy(
                            sc_stage[:, j2 * C_IN:(j2 + 1) * C_IN], tp_ps[:])
                    cc_in = dram.tile([VLOC, C_IN], mm_dt, name=f"ccin{k}")
                    cc_out = dram.tile([V, C_IN], mm_dt, name=f"ccout{k}")
                    nc.scalar.dma_start(
                        cc_in.rearrange("(p j) c -> p j c", p=P),
                        sc_stage[:].rearrange("p (j c) -> p j c",
                                              j=VLOC // P))
                    nc.gpsimd.collective_compute(
                        "AllGather",
                        mybir.AluOpType.bypass,
                        replica_groups=[list(range(N_CORES))],
                        ins=[cc_in.opt()],
                        outs=[cc_out.opt()],
                    )
                    cc_r = cc_out.rearrange("(g p j) c -> g p j c",
                                            p=P, j=NB_S)
                    sk_tiles = load_stat(cc_r, k)

            # ---- out[o, v] = sum_k w_k^T @ T_k + bias ----
            for ch in range(N_CH):
                sl = slice(ch * 512, (ch + 1) * 512)
                ein = accp.tile([C_OUT, 512], f32, name=f"ein{ch}", tag="acc")
                for k in range(K_CHEB):
                    nc.tensor.matmul(
                        ein[:],
                        lhsT=w_sb[:, k * C_OUT:(k + 1) * C_OUT],
                        rhs=t_tiles[k][:, sl],
                        start=(k == 0),
                        stop=(k == K_CHEB - 1),
                    )
                res = work.tile([C_OUT, 512], f32, name=f"res{ch}", tag="res")
                nc.vector.tensor_scalar_add(res[:], ein[:], bias_sb[:])
                nc.scalar.dma_start(out.ap()[:, sl], res[:])

    nc.compile()
    return nc


def _interleave_rows(a, nb):
    """Within each nb*128-row group, reorder rows so row g*G+nb*p+u holds
    original row g*G+u*128+p (one contiguous per-partition read)."""
    ng = a.shape[0] // (P * nb)
    return np.ascontiguousarray(
        a.reshape(ng, nb, P, a.shape[1]).transpose(0, 2, 1, 3)
        .reshape(a.shape))


def _prep_inputs(x, L, weights, bias, cfg: str):
    np_dt = ml_dtypes.bfloat16 if cfg == "bf16" else np.float32
    x = np.asarray(x, dtype=np.float32)
    L = np.asarray(L, dtype=np.float32)
    weights = np.asarray(weights, dtype=np.float32)
    bias = np.asarray(bias, dtype=np.float32)

    Lt = np.ascontiguousarray(L.T).astype(np_dt)          # (V, V)
    xt = _interleave_rows(
        np.ascontiguousarray(x.T).astype(np_dt), NB_S)    # (V, C_IN)

    wf = np.ascontiguousarray(weights.reshape(K_CHEB * C_IN, C_OUT))
    b_ = np.ascontiguousarray(bias.reshape(C_OUT, 1))
    id16 = np.eye(C_IN, dtype=np.float32)

    in_maps = []
    for d in range(N_CORES):
        cols = slice(d * VLOC, (d + 1) * VLOC)
        in_maps.append({
            "lt": _interleave_rows(np.ascontiguousarray(Lt[:, cols]), NB),
            "xt": xt,
            "xc": np.ascontiguousarray(x[:, cols]),
            "wf": wf,
            "bias_in": b_,
            "id16": id16,
        })
    return in_maps


def run(x, L, weights, bias, cfg: str = "bf16", trace: bool = False):
    if cfg not in _CACHE:
        _CACHE[cfg] = _build(cfg)
    nc = _CACHE[cfg]
    in_maps = _prep_inputs(x, L, weights, bias, cfg)
    res = bass_utils.run_bass_kernel_spmd(
        nc, in_maps, core_ids=list(range(N_CORES)), trace=trace)
    out = np.concatenate([res.results[d]["out"] for d in range(N_CORES)],
                         axis=1)
    return out.astype(np.float32), res


def kernel(x, L, weights, bias):
    out, _ = run(x, L, weights, bias, cfg="bf16")
    return out
